# revision 35
# baseline (speedup 1.0000x reference)
"""Deformable temporal conv1d (kernel (1,3), stride 1, pad 1) on 8 TRN2 cores.

Algorithm (fp16 compute, fp32 accumulation in PSUM):
  offset = conv1x3(x, off_w) + off_b            (6 ch) -> dy_j, dx_j per tap
  mask   = sigmoid(conv1x3(x, mask_w) + mask_b) (3 ch)
  z_j    = W_j @ x   (1x1 channel mix per tap, W_j = conv_w[:,:,0,j])
  out[o,h,w] = conv_b[o] + sum_j sum_{dh,dw} A_{j,dh,dw}[h,w]
                                   * z_j[o, h+dh, w+(j-1)+dw]
  with A = mask_j * relu(1-|dy_j-dh|) * relu(1-|dx_j-dw|)   (exact bilinear)

The main pass uses the 3x3 (dh,dw) window (exact when |offsets|<1).  A
per-block runtime flag detects |off|>1 and triggers an If-gated outer-ring
pass (|dh|=2 or |dw|=2, 36 extra terms) making the result exact for
|offsets|<2.  Zero padding outside the image comes from zero halo rows and
3 zero columns on each side of the 134-pitch slabs.

Per (term, chunk) the A map is expanded across partitions by a PE rank-1
matmul (fp16 moving operand), evacuated PSUM->SBUF fp16 by the scalar
engine, then fp16 tensor_tensor mult+add on DVE (2x packed mode).  Each
chunk keeps two independent accumulator chains (merged at finalize) to
halve the serial add-chain depth.  gpsimd offload knobs exist but are
disabled: measured pool TT/partition_broadcast throughput (~2 ns/elem)
poisons the term pipeline.

Sharding: core i handles batch b=i//2, H-half hh=i%2 (256 output rows), with
2 halo rows on each side supplied by the host (zeros at image edges).
"""

import numpy as np
from contextlib import ExitStack

import concourse.bass as bass
import concourse.bacc as bacc
import concourse.tile as tile
import concourse.mybir as mybir
from concourse.bass_utils import run_bass_kernel_spmd

F32 = mybir.dt.float32
F16 = mybir.dt.float16
AF = mybir.ActivationFunctionType
OP = mybir.AluOpType

B, C, H, W = 4, 128, 512, 128
NCORES = 8
ROWS = H // 2          # output rows per core
PITCH = W + 6          # slab pitch: cols [0..2] and [131..133] are zero
KTAP = 3
HALO = 2               # halo rows on each side (ring reads dh=+-2)

R = 32                 # output rows per block
NB = ROWS // R         # 8 blocks
NPX = R * W            # 4096 pixels per block
SROWS = R + 2 * HALO   # 36 slab rows per block
CH = 2048              # modulation chunk (pixels)
NCHB = NPX // CH       # 2 chunks per block
CROWS = CH // W        # 16 output rows per chunk
PCH = 1024             # PSUM broadcast sub-chunk (pixels)

# (dh, dw) ring combos: |dh|=2 x dw in {-1,0,1}  +  dh in {-1,0,1} x |dw|=2
RING = [(dh, dw) for dh in (-2, 2) for dw in (-1, 0, 1)] + \
       [(dh, dw) for dh in (-1, 0, 1) for dw in (-2, 2)]

# terms whose accumulate runs on gpsimd instead of DVE (tuning knob);
# measured: pool TT runs at 0.42 efficiency (~2.5us per 1024-px chunk) and
# sits on the serial accumulator chain -> keep empty.
POOL_ADD_TERMS = frozenset()
# terms whose A-row expansion runs on gpsimd partition_broadcast (SBUF to
# SBUF, bypassing the PE broadcast + PSUM + scalar-engine evacuation)
POOL_PBCAST_TERMS = frozenset()
# terms whose product runs on gpsimd tensor_tensor (off the acc chain)
POOL_MULT_TERMS = frozenset()


def build_nc():
    nc = bacc.Bacc()
    x_d = nc.declare_dram_parameter("x", [C, ROWS + 2 * HALO, W], F16,
                                    isOutput=False)
    wz_d = nc.declare_dram_parameter("wz", [C, KTAP * C], F16, isOutput=False)
    wom_d = nc.declare_dram_parameter("wom", [C, KTAP * 9], F16,
                                      isOutput=False)
    ob_d = nc.declare_dram_parameter("ob", [9, 1], F32, isOutput=False)
    cb_d = nc.declare_dram_parameter("cb", [C, 1], F32, isOutput=False)
    ones_d = nc.declare_dram_parameter("ones", [1, C], F16, isOutput=False)
    b5_d = nc.declare_dram_parameter("b5", [C, 5], F32, isOutput=False)
    out_d = nc.declare_dram_parameter("out", [C, ROWS * W], F16,
                                      isOutput=True)

    with tile.TileContext(nc) as tc, ExitStack() as ctx:
        cpool = ctx.enter_context(tc.tile_pool(name="consts", bufs=1))
        ompool = ctx.enter_context(tc.tile_pool(name="om", bufs=2))
        mpool = ctx.enter_context(tc.tile_pool(name="maps", bufs=2))
        ampool = ctx.enter_context(tc.tile_pool(name="amaps", bufs=4))
        rampool = ctx.enter_context(tc.tile_pool(name="ramaps", bufs=4))
        rpool = ctx.enter_context(tc.tile_pool(name="mrows", bufs=2))
        apool = ctx.enter_context(tc.tile_pool(name="acc", bufs=NCHB))
        tpool = ctx.enter_context(tc.tile_pool(name="tmp", bufs=3))
        aspool = ctx.enter_context(tc.tile_pool(name="asb", bufs=3))
        opool = ctx.enter_context(tc.tile_pool(name="ostg", bufs=2))
        spool = ctx.enter_context(tc.tile_pool(name="stage", bufs=3))
        fpool = ctx.enter_context(tc.tile_pool(name="flag", bufs=2))
        ps_om = ctx.enter_context(
            tc.tile_pool(name="ps_om", bufs=1, space="PSUM"))
        ps_z = ctx.enter_context(
            tc.tile_pool(name="ps_z", bufs=2, space="PSUM"))
        ps_a = ctx.enter_context(
            tc.tile_pool(name="ps_a", bufs=2, space="PSUM"))

        # constants
        wz = cpool.tile([C, KTAP * C], F16, tag="wz")
        nc.sync.dma_start(wz[:], wz_d[:])
        wom = cpool.tile([C, KTAP * 9], F16, tag="wom")
        nc.sync.dma_start(wom[:], wom_d[:])
        ob = cpool.tile([9, 1], F32, tag="ob")
        nc.sync.dma_start(ob[:], ob_d[:])
        cb = cpool.tile([C, 1], F32, tag="cb")
        nc.sync.dma_start(cb[:], cb_d[:])
        ones = cpool.tile([1, C], F16, tag="ones")
        nc.sync.dma_start(ones[:], ones_d[:])
        # b5 columns hold -dlt for dlt in (-2,-1,0,1,2): (2,1,0,-1,-2)
        b5 = cpool.tile([C, 5], F32, tag="b5")
        nc.sync.dma_start(b5[:], b5_d[:])

        # persistent slabs (zero columns cleared once)
        xs_bufs = []
        z_bufs = []
        for i in range(2):
            xs = cpool.tile([C, SROWS, PITCH], F16, tag=f"xsbuf{i}")
            nc.gpsimd.memset(xs[:, :, 0:3], 0.0)
            nc.gpsimd.memset(xs[:, :, PITCH - 3:PITCH], 0.0)
            xs_bufs.append(xs)
            zrow = []
            for j in range(KTAP):
                z = cpool.tile([C, SROWS, PITCH], F16, tag=f"zbuf{i}_{j}")
                nc.gpsimd.memset(z[:, :, 0:3], 0.0)
                nc.gpsimd.memset(z[:, :, PITCH - 3:PITCH], 0.0)
                zrow.append(z)
            z_bufs.append(zrow)

        def bcast(aps, row_ap):
            """Expand row_ap [1, PCH] (fp16, partition 0) across 128
            partitions into PSUM aps [C, PCH] via rank-1 matmuls (<=512
            cols per PSUM bank)."""
            for hf in range(PCH // 512):
                nc.tensor.matmul(
                    aps[:, hf * 512:(hf + 1) * 512],
                    ones[:], row_ap[:, hf * 512:(hf + 1) * 512],
                    start=True, stop=True)

        def expand(stg, chm, pool_route=False):
            """Stage chunk chm of an A row into an SBUF fp16 tile
            [C, CROWS, W]: either PE broadcast + scalar-engine PSUM
            evacuation, or gpsimd partition_broadcast (SBUF->SBUF)."""
            asb = aspool.tile([C, CROWS, W], F16, tag="asb", name="asb")
            flat = asb[:].rearrange("p a b -> p (a b)")
            if pool_route:
                nc.gpsimd.partition_broadcast(
                    flat, stg[:, chm * CH:(chm + 1) * CH])
                return asb
            for sc in range(CH // PCH):
                aps = ps_a.tile([C, PCH], F32, tag="aps", name="aps")
                o = chm * CH + sc * PCH
                bcast(aps, stg[:, o:o + PCH])
                nc.scalar.copy(flat[:, sc * PCH:(sc + 1) * PCH], aps[:])
            return asb

        for b in range(NB):
            r0 = b * R
            xs = xs_bufs[b % 2]
            # bulk x load on the gpsimd DMA queue so the SP ring stays free
            # for the latency-critical per-term stg/mrows transfers
            nc.gpsimd.dma_start(xs[:, :, 3:3 + W], x_d[:, r0:r0 + SROWS, :])

            # ---- offset/mask conv over the R output rows ----
            om = ompool.tile([9, NPX], F16, tag="om")
            for g in range(NPX // 512):
                ps = ps_om.tile([9, 512], F32, tag="psom")
                sr = HALO + g * 4
                nc.tensor.matmul(
                    ps[:, 0:1], ones[0:1, 0:9], xs[0:1, sr, 0:1],
                    start=True, stop=True, skip_group_check=True)
                for t in range(KTAP):
                    nc.tensor.matmul(
                        ps[:],
                        wom[:, t * 9:(t + 1) * 9],
                        xs[:, sr:sr + 4, 2 + t:2 + t + W],
                        start=(t == 0), stop=(t == KTAP - 1))
                nc.scalar.activation(om[:, g * 512:(g + 1) * 512], ps[:],
                                     AF.Identity, bias=ob[:])

            # ---- maps: [128, 9, bw], pixel p = part*bw + f ----
            bw = NPX // 128
            m9 = mpool.tile([C, 9, bw], F16, tag="m9")
            for m in range(9):
                nc.sync.dma_start(m9[:, m, :], om[m:m + 1, :])
            dy3 = m9[:, 0:6:2, :]
            dx3 = m9[:, 1:7:2, :]
            msk3 = mpool.tile([C, 3, bw], F32, tag="msk3")
            nc.scalar.activation(msk3[:], m9[:, 6:9, :], AF.Sigmoid)

            wy = {}
            wx = {}
            ay = {}
            ax = {}
            for i, dlt in ((1, -1.0), (2, 0.0), (3, 1.0)):
                nbias = b5[:, i:i + 1]
                ayt = mpool.tile([C, 3, bw], F32, tag=f"ay{i}")
                nc.scalar.activation(ayt[:], dy3, AF.Abs, bias=nbias)
                wyt = mpool.tile([C, 3, bw], F32, tag=f"wy{i}")
                nc.scalar.activation(wyt[:], ayt[:], AF.Relu, bias=1.0,
                                     scale=-1.0)
                ay[dlt] = ayt
                wy[dlt] = wyt
                axt = mpool.tile([C, 3, bw], F32, tag=f"ax{i}")
                nc.scalar.activation(axt[:], dx3, AF.Abs, bias=nbias)
                wxt = mpool.tile([C, 3, bw], F32, tag=f"wx{i}")
                nc.scalar.activation(wxt[:], axt[:], AF.Relu, bias=1.0,
                                     scale=-1.0)
                ax[dlt] = axt
                wx[dlt] = wxt

            # ring flag: any |dy|>1 or |dx|>1 in this block?
            mxf = fpool.tile([C, 3 * bw], F32, tag="mxf")
            nc.vector.tensor_tensor(
                mxf[:], ay[0.0][:, :, :].rearrange("p a b -> p (a b)"),
                ax[0.0][:, :, :].rearrange("p a b -> p (a b)"), op=OP.max)
            rmx = fpool.tile([C, 1], F32, tag="rmx")
            nc.vector.reduce_max(rmx[:], mxf[:], axis=mybir.AxisListType.X)
            rmxT = fpool.tile([1, C], F32, tag="rmxT")
            nc.sync.dma_start(rmxT[:], rmx[:])
            rfl = fpool.tile([1, 1], F32, tag="rfl")
            nc.vector.reduce_max(rfl[:], rmxT[:], axis=mybir.AxisListType.X)
            rfl01 = fpool.tile([1, 1], F32, tag="rfl01")
            nc.vector.tensor_scalar(rfl01[:], rfl[:], 1.0, None, op0=OP.is_gt)

            # ---- A maps for the 3x3 window -> mrows rows 0..26 (fp16);
            # parking them as rows decouples the partition-gather DMA from
            # the per-term critical path ----
            mrows = rpool.tile([27 + 36, NPX], F16, tag="mrows")
            myw = {}
            for ih, dh in enumerate((-1.0, 0.0, 1.0)):
                mywt = mpool.tile([C, 3, bw], F32, tag=f"myw{ih}")
                nc.vector.tensor_tensor(mywt[:], msk3[:], wy[dh][:],
                                        op=OP.mult)
                myw[dh] = mywt
                for iw, dw in enumerate((-1.0, 0.0, 1.0)):
                    amap = ampool.tile([C, 3, bw], F16, tag="amap",
                                       name=f"amap{ih}{iw}")
                    nc.vector.tensor_tensor(amap[:], mywt[:], wx[dw][:],
                                            op=OP.mult)
                    for j in range(KTAP):
                        t = ih * 9 + iw * 3 + j
                        nc.sync.dma_start(mrows[t:t + 1, :], amap[:, j, :])

            # ---- z convs over all slab rows ----
            zt = z_bufs[b % 2]
            for j in range(KTAP):
                z = zt[j]
                for g in range(SROWS // 4):
                    ps = ps_z.tile([C, 512], F32, tag="psz")
                    nc.tensor.matmul(
                        ps[:], wz[:, j * C:(j + 1) * C],
                        xs[:, g * 4:(g + 1) * 4, 3:3 + W],
                        start=True, stop=True)
                    nc.scalar.copy(z[:, g * 4:(g + 1) * 4, 3:3 + W], ps[:])

            # ---- main modulation: 27 terms x 4 chunks (term-outer so each
            # A row is staged to partition 0 once per block) ----
            # two independent accumulator chains per chunk (halves the
            # serial add-chain depth); merged before finalize
            accs = [apool.tile([C, CROWS, W], F16, tag="acc", name=f"acc{c}")
                    for c in range(NCHB)]
            accs2 = [apool.tile([C, CROWS, W], F16, tag="acc2",
                                name=f"acc2{c}")
                     for c in range(NCHB)]
            for ih, dh in enumerate((-1, 0, 1)):
                for iw, dw in enumerate((-1, 0, 1)):
                    for j in range(KTAP):
                        t = ih * 9 + iw * 3 + j
                        stg = spool.tile([1, NPX], F16, tag="stg")
                        nc.sync.dma_start(stg[:], mrows[t:t + 1, :])
                        for chm in range(NCHB):
                            or0 = chm * CROWS
                            acc = accs[chm] if t < 14 else accs2[chm]
                            asb = expand(stg, chm,
                                         pool_route=t in POOL_PBCAST_TERMS)
                            zsrc = zt[j][:,
                                         HALO + or0 + dh:
                                         HALO + or0 + dh + CROWS,
                                         3 + (j - 1) + dw:
                                         3 + (j - 1) + dw + W]
                            if t in (0, 14):
                                nc.vector.tensor_tensor(
                                    acc[:], asb[:], zsrc, op=OP.mult)
                            else:
                                tmp = tpool.tile([C, CROWS, W], F16,
                                                 tag="tmp")
                                if t in POOL_MULT_TERMS:
                                    nc.gpsimd.tensor_tensor(
                                        tmp[:], asb[:], zsrc, op=OP.mult)
                                else:
                                    nc.vector.tensor_tensor(
                                        tmp[:], asb[:], zsrc, op=OP.mult)
                                if t in POOL_ADD_TERMS:
                                    nc.gpsimd.tensor_add(acc[:], acc[:],
                                                         tmp[:])
                                else:
                                    nc.vector.tensor_add(acc[:], acc[:],
                                                         tmp[:])

            # ---- ring pass (rare): 36 extra terms, If-gated ----
            flag_regs = []
            for et in (mybir.EngineType.PE, mybir.EngineType.Activation,
                       mybir.EngineType.DVE, mybir.EngineType.SP):
                eng = nc.engines[et]
                r = eng.alloc_register(f"ringflag{b}")
                eng.reg_load(r, rfl01[:].bitcast(mybir.dt.uint32))
                flag_regs.append(r)
            cond = nc.snap(bass.RegisterHandles(flag_regs), donate=True)
            with tc.If(cond != 0):
                for i, dlt in ((0, -2.0), (4, 2.0)):
                    nbias = b5[:, i:i + 1]
                    ayt = mpool.tile([C, 3, bw], F32, tag=f"ray{i}")
                    nc.scalar.activation(ayt[:], dy3, AF.Abs, bias=nbias)
                    wyt = mpool.tile([C, 3, bw], F32, tag=f"rwy{i}")
                    nc.scalar.activation(wyt[:], ayt[:], AF.Relu, bias=1.0,
                                         scale=-1.0)
                    wy[dlt] = wyt
                    axt = mpool.tile([C, 3, bw], F32, tag=f"rax{i}")
                    nc.scalar.activation(axt[:], dx3, AF.Abs, bias=nbias)
                    wxt = mpool.tile([C, 3, bw], F32, tag=f"rwx{i}")
                    nc.scalar.activation(wxt[:], axt[:], AF.Relu, bias=1.0,
                                         scale=-1.0)
                    wx[dlt] = wxt
                for i, dh in enumerate((-2.0, 2.0)):
                    mywt = mpool.tile([C, 3, bw], F32, tag=f"rmyw{i}")
                    nc.vector.tensor_tensor(mywt[:], msk3[:], wy[dh][:],
                                            op=OP.mult)
                    myw[dh] = mywt
                for ti, (dh, dw) in enumerate(RING):
                    amap = rampool.tile([C, 3, bw], F16, tag="ramap",
                                        name=f"ramap{ti}")
                    nc.vector.tensor_tensor(
                        amap[:], myw[float(dh)][:], wx[float(dw)][:],
                        op=OP.mult)
                    for j in range(KTAP):
                        t = 27 + ti * 3 + j
                        nc.sync.dma_start(mrows[t:t + 1, :], amap[:, j, :])
                for ti, (dh, dw) in enumerate(RING):
                    for j in range(KTAP):
                        t = 27 + ti * 3 + j
                        stg = spool.tile([1, NPX], F16, tag="stg")
                        nc.sync.dma_start(stg[:], mrows[t:t + 1, :])
                        for chm in range(NCHB):
                            or0 = chm * CROWS
                            acc = accs[chm]
                            asb = expand(stg, chm)
                            zsrc = zt[j][:,
                                         HALO + or0 + dh:
                                         HALO + or0 + dh + CROWS,
                                         3 + (j - 1) + dw:
                                         3 + (j - 1) + dw + W]
                            tmp = tpool.tile([C, CROWS, W], F16, tag="tmp")
                            nc.vector.tensor_tensor(
                                tmp[:], asb[:], zsrc, op=OP.mult)
                            nc.vector.tensor_add(acc[:], acc[:], tmp[:])

            # ---- finalize: merge chains + bias add + store ----
            for chm in range(NCHB):
                nc.vector.tensor_add(accs[chm][:], accs[chm][:],
                                     accs2[chm][:])
                ost = opool.tile([C, CROWS, W], F16, tag="ost")
                nc.scalar.activation(ost[:], accs[chm][:], AF.Identity,
                                     bias=cb[:])
                p0 = (r0 + chm * CROWS) * W
                nc.scalar.dma_start(out_d[:, p0:p0 + CH], ost[:])
    return nc


def prep_inputs(x, conv_w, conv_b, off_w, off_b, mask_w, mask_b):
    x = np.ascontiguousarray(x, np.float32)
    conv_w = np.asarray(conv_w, np.float32)
    wz = np.ascontiguousarray(
        np.concatenate([conv_w[:, :, 0, j].T for j in range(KTAP)], axis=1),
        np.float16)
    wom_t = []
    for t in range(KTAP):
        m = np.concatenate([np.asarray(off_w)[:, :, 0, t],
                            np.asarray(mask_w)[:, :, 0, t]], axis=0)
        wom_t.append(m.T)
    wom = np.ascontiguousarray(np.concatenate(wom_t, axis=1), np.float16)
    ob = np.concatenate([off_b, mask_b]).reshape(9, 1).astype(np.float32)
    cb = np.asarray(conv_b, np.float32).reshape(C, 1)
    ones = np.ones((1, C), np.float16)
    b5 = np.tile(np.array([[2.0, 1.0, 0.0, -1.0, -2.0]], np.float32), (C, 1))

    xp = np.pad(x, ((0, 0), (0, 0), (HALO, HALO), (0, 0))).astype(np.float16)
    halves = H // ROWS
    in_maps = []
    for i in range(NCORES):
        b, hh = i // halves, i % halves
        xs = np.ascontiguousarray(
            xp[b, :, hh * ROWS:hh * ROWS + ROWS + 2 * HALO, :])
        in_maps.append({"x": xs, "wz": wz, "wom": wom, "ob": ob, "cb": cb,
                        "ones": ones, "b5": b5})
    return in_maps


_NC_CACHE = {}


def kernel(x, conv_w, conv_b, off_w, off_b, mask_w, mask_b, **run_kw):
    if "nc" not in _NC_CACHE:
        _NC_CACHE["nc"] = build_nc()
    nc = _NC_CACHE["nc"]
    if not nc.is_finalized():
        nc.finalize()
    in_maps = prep_inputs(x, conv_w, conv_b, off_w, off_b, mask_w, mask_b)
    res = run_bass_kernel_spmd(nc, in_maps, list(range(NCORES)), **run_kw)
    out = np.empty((B, C, H, W), np.float32)
    halves = H // ROWS
    for i in range(NCORES):
        b, hh = i // halves, i % halves
        out[b, :, hh * ROWS:(hh + 1) * ROWS, :] = \
            res.results[i]["out"].astype(np.float32).reshape(C, ROWS, W)
    _NC_CACHE["last_result"] = res
    return out
